# revision 72
# baseline (speedup 1.0000x reference)
"""GAT message-passing kernel for Trainium2, 8 NeuronCores.

Problem (see harness reference): for each head h:
    Wh   = x @ W[h]                                  [B,N,F]
    e    = leaky_relu((Wh@a_src)[:,:,None] + (Wh@a_dst)[:,None,:], 0.2)
    att  = exp(where(adj>0, e, -9e15)) * big_w        [B,N,N]
    att /= clip(sum(att, axis=1), 1e-12)              (column L1 norm)
    out_h = elu(att @ Wh)
    out   = concat over heads                         [B,N,H*F]

big_w is bipartite: nonzero only on blocks (i<U, j>=U) [= weights.T] and
(i>=U, j<U) [= weights], so att has two independent 1024x1024 nonzero
blocks; block A fully determines out rows [0,U), block B rows [U,2N).

Sharding: core c -> (b = c//4, block = (c//2)%2, head-pair = c%2).
Each core computes one block for two heads. All block math is in the
transposed [j, i] layout (j = contraction node on partitions):
  att^T[j,i] = exp(lrelu(s[i] + d[j])) * adjT[j,i] * wmx[j,i]
The host supplies adjT (pre-transposed adjacency block) and wmx (wm for
block A, wm.T for block B) so NO device-side transposes are needed; x
is passed pre-transposed per node half for the same reason. The column
denominator is a free-axis fused reduce (accum_out); 1/denom folds into
per-row scaling of Wh (computed lazily per tile, scaled straight out of
PSUM). The output is produced transposed ([f, i], one 512-wide matmul
per j-tile accumulating into persistent PSUM) and un-transposed on the
host.

Engine split: Act does Prelu/Exp, DVE does mask-mult+denom / scaling /
elu pieces, Pool does the adj*w merge + broadcasts + stores, PE does all
matmuls. adjT/wmx stream on the sync DMA ring; phase-0 loads go first on
the scalar ring so they never queue behind the 8 MB adjacency stream.
"""

import threading
import numpy as np

B, N, FIN, F, H, U = 2, 2048, 128, 128, 4, 1024
P = 128
JT = U // P    # 8 tiles per block axis
ALPHA = 0.2

TRACE = False          # set by test.py for profiling runs
LAST_EXEC_NS = None    # exec_time_ns of the last traced run
_BUILD_LOCK = threading.Lock()
_CACHE = {}


def _build_program():
    from concourse import bacc
    import concourse.mybir as mybir
    import concourse.tile as tile

    dt = mybir.dt
    Alu = mybir.AluOpType
    Act = mybir.ActivationFunctionType

    nc = bacc.Bacc("TRN2", target_bir_lowering=False, debug=False, num_devices=8)

    adjT = nc.dram_tensor("adjT", [U, U], dt.int32, kind="ExternalInput")
    wmx = nc.dram_tensor("wmx", [U, U], dt.float32, kind="ExternalInput")
    xiT = nc.dram_tensor("xiT", [FIN, U], dt.float32, kind="ExternalInput")
    xjT = nc.dram_tensor("xjT", [FIN, U], dt.float32, kind="ExternalInput")
    # packed per-head-pair params: cols [0:256)=W (h-major), [256:512)=W^T,
    # [512:516)=a columns (h0src, h0dst, h1src, h1dst). One fat-row DMA.
    wcomb = nc.dram_tensor("wcomb", [FIN, 2 * F + 2 * FIN + 4], dt.float32, kind="ExternalInput")
    outh = nc.dram_tensor("outh", [2 * F, U], dt.float32, kind="ExternalOutput")

    with tile.TileContext(nc) as tc:
        with (
            tc.tile_pool(name="persist", bufs=1) as persist,
            tc.tile_pool(name="abfp", bufs=8) as abfp,
            tc.tile_pool(name="wbfp", bufs=8) as wbfp,
            tc.tile_pool(name="adjwp", bufs=3) as adjwp,
            tc.tile_pool(name="attp", bufs=4) as attp,
            tc.tile_pool(name="lrp", bufs=2) as lrp,
            tc.tile_pool(name="ep", bufs=3) as ep,
            tc.tile_pool(name="elup", bufs=2) as elup,
            tc.tile_pool(name="ps_small", bufs=1, space="PSUM") as ps_small,
            tc.tile_pool(name="ps_wh", bufs=1, space="PSUM") as ps_wh,
            tc.tile_pool(name="ps_acc", bufs=1, space="PSUM") as ps_acc,
        ):
            hp_ctx = tc.high_priority()
            hp_ctx.__enter__()
            # -------- phase-0 loads first on the scalar ring (tiny, must not
            # queue behind the adjacency stream); smallest/most-blocking first
            # split across the two HWDGE rings so they land in parallel
            comb = persist.tile([P, 2 * F + 2 * FIN + 4], dt.float32)
            nc.scalar.dma_start(out=comb, in_=wcomb[:, :])
            xj_f = persist.tile([P, U], dt.float32)
            nc.sync.dma_start(out=xj_f, in_=xjT[:, :])
            xi_f = persist.tile([P, U], dt.float32)
            nc.sync.dma_start(out=xi_f, in_=xiT[:, :])
            at0 = persist.tile([P, U], dt.int32)
            nc.sync.dma_start(out=at0, in_=adjT[0:P, :])
            wt0 = persist.tile([P, U], dt.float32)
            nc.sync.dma_start(out=wt0, in_=wmx[0:P, :])

            # f32r copies: full-rate PE fp32 (the plain-f32 4-pass matmuls run
            # ~3us each at the cold low p-state and gate the whole phase 0)
            comb_r = persist.tile([P, 2 * F + 2 * FIN + 4], dt.float32r)
            nc.vector.tensor_copy(comb_r, comb)
            xi_r = persist.tile([P, U], dt.float32r)
            nc.vector.tensor_copy(xi_r, xi_f)
            xj_r = persist.tile([P, U], dt.float32r)
            nc.vector.tensor_copy(xj_r, xj_f)

            def wp_h(h):
                return comb_r[:, h * F : (h + 1) * F]

            def wpT_h(h):
                return comb_r[:, 2 * F + h * FIN : 2 * F + (h + 1) * FIN]

            # -------- phase 0: scores. s_bc is produced ALREADY BROADCAST by
            # the PE: lhsT = wa_src replicated across 128 columns makes every
            # output partition the same s row. No partition_broadcast, no
            # [1,512] row copies — far shorter critical path to the first Prelu.
            ones_t = persist.tile([P, P], dt.float32)
            nc.gpsimd.memset(ones_t, 1.0)
            # wa for both heads first, then all s matmuls, then all d matmuls:
            # PE executes in order, and the first Prelu needs s_bc0+d_cols0.
            was, reps = [], []
            for h in range(2):
                wa_ps = ps_wh.tile([P, F], dt.float32, tag="wh")
                nc.tensor.matmul(
                    wa_ps[:, 0:2],
                    wpT_h(h), comb_r[:, 2 * F + 2 * FIN + 2 * h : 2 * F + 2 * FIN + 2 * h + 2],
                    start=True, stop=True,
                )
                wa = persist.tile([P, 2], dt.float32, tag=f"wa{h}", name=f"wa{h}")
                nc.vector.tensor_copy(wa, wa_ps[:, 0:2])
                wa_r = persist.tile([P, 2], dt.float32r, tag=f"war2{h}", name=f"war2{h}")
                nc.vector.tensor_copy(wa_r, wa_ps[:, 0:2])
                was.append(wa_r)
                wa_rep = persist.tile([P, P], dt.float32r, tag=f"war{h}", name=f"war{h}")
                nc.vector.tensor_scalar(
                    out=wa_rep, in0=ones_t, scalar1=wa[:, 0:1], scalar2=None,
                    op0=Alu.mult,
                )
                reps.append(wa_rep)

            s_bc = []
            for h in range(2):
                sb = persist.tile([P, U], dt.float32, tag=f"sbc{h}", name=f"sbc{h}")
                for q in range(2):
                    s_ps = ps_small.tile([P, 512], dt.float32, tag="sb", bufs=2)
                    nc.tensor.matmul(
                        s_ps, reps[h], xi_r[:, q * 512 : (q + 1) * 512],
                        start=True, stop=True,
                    )
                    nc.vector.tensor_copy(sb[:, q * 512 : (q + 1) * 512], s_ps)
                s_bc.append(sb)

            d_cols = []
            for h in range(2):
                pd = ps_small.tile([P, 2 * JT], dt.float32, tag="d")
                for t in range(JT):
                    nc.tensor.matmul(
                        pd[:, 2 * t : 2 * t + 2],
                        xj_r[:, t * P : (t + 1) * P], was[h],
                        start=True, stop=True,
                    )
                dc = persist.tile([P, JT], dt.float32, tag=f"dcol{h}", name=f"dcol{h}")
                nc.vector.tensor_copy(
                    dc, pd.rearrange("p (n two) -> p n two", two=2)[:, :, 1:2]
                )
                d_cols.append(dc)
            hp_ctx.__exit__(None, None, None)

            # -------- adj and wm stream in as bf16 via gpsimd cast-DMAs
            # (SWDGE converts dtypes in flight), so the adj*w merge runs as an
            # all-bf16 tensor_tensor at 2x DVE rate instead of a 1x STT.
            abf_t, wbf_t = [None], [None]
            for k in range(1, JT):
                ab = abfp.tile([P, U], dt.bfloat16)
                nc.gpsimd.dma_start(out=ab, in_=adjT[k * P : (k + 1) * P, :])
                wb = wbfp.tile([P, U], dt.bfloat16)
                nc.gpsimd.dma_start(out=wb, in_=wmx[k * P : (k + 1) * P, :])
                abf_t.append(ab)
                wbf_t.append(wb)


            whs = []
            for h in range(2):
                ws_t = persist.tile([P, JT, F], dt.bfloat16, tag=f"whs{h}", name=f"whs{h}")
                whs.append(ws_t)
            den = persist.tile([P, 2 * JT], dt.float32)   # col = 2k + h
            rec = persist.tile([P, 2 * JT], dt.float32)

            o_ps = []
            for h in range(2):
                row = []
                for c in range(2):
                    o_t = ps_acc.tile(
                        [P, 512], dt.float32, tag=f"o{h}{c}", name=f"o{h}{c}"
                    )
                    row.append(o_t)
                o_ps.append(row)

            # -------- per j-tile: adjw merge (2x bf16), attention, denom,
            # matmul accumulate
            for k in range(JT):
                aw = adjwp.tile([P, U], dt.bfloat16)
                if k == 0:
                    # tile 0 came over the fast sync ring; merge with a 1x STT
                    # so the DVE queue never stalls on the SWDGE stream
                    nc.vector.scalar_tensor_tensor(
                        out=aw, in0=at0, scalar=1.0, in1=wt0,
                        op0=Alu.mult, op1=Alu.mult,
                    )
                else:
                    nc.vector.tensor_tensor(out=aw, in0=abf_t[k], in1=wbf_t[k], op=Alu.mult)
                att2 = []
                for h in range(2):
                    lr = lrp.tile([P, U], dt.float32, tag="lr")
                    nc.scalar.activation(
                        lr, s_bc[h], Act.Prelu,
                        bias=d_cols[h][:, k : k + 1], scale=1.0, alpha=ALPHA,
                    )
                    e = ep.tile([P, U], dt.bfloat16, tag="e")
                    nc.scalar.activation(e, lr, Act.Exp)
                    att = attp.tile([P, U], dt.bfloat16)
                    nc.vector.scalar_tensor_tensor(
                        out=att, in0=e, scalar=1.0, in1=aw,
                        op0=Alu.mult, op1=Alu.mult,
                        accum_out=den[:, 2 * k + h : 2 * k + h + 1],
                    )
                    att2.append(att)
                # no 1e-12 clip needed: every row has hundreds of positive
                # terms (e > 0 everywhere, ~half the adj entries are 1)
                nc.vector.reciprocal(rec[:, 2 * k : 2 * k + 2], den[:, 2 * k : 2 * k + 2])
                for h in range(2):
                    wh_ps = ps_wh.tile([P, F], dt.float32, tag="wh")
                    nc.tensor.matmul(
                        wh_ps, xj_r[:, k * P : (k + 1) * P], wp_h(h),
                        start=True, stop=True,
                    )
                    nc.vector.tensor_scalar(
                        out=whs[h][:, k, :], in0=wh_ps,
                        scalar1=rec[:, 2 * k + h : 2 * k + h + 1],
                        scalar2=None, op0=Alu.mult,
                    )
                    for c in range(2):
                        nc.tensor.matmul(
                            o_ps[h][c],
                            whs[h][:, k, :],
                            att2[h][:, c * 512 : (c + 1) * 512],
                            start=(k == 0),
                            stop=(k == JT - 1),
                        )

            # -------- elu + store (transposed [f, i]; host un-transposes)
            oT_sb = persist.tile([P, 2, U], dt.float32)
            for h in range(2):
                for c in range(2):
                    # elu(y) = relu(y) + exp(min(y,0)) - 1
                    #        = relu(y) + (min(exp(y), 1) - 1)   (exp monotone;
                    #          exp overflow to inf is absorbed by the min)
                    src = o_ps[h][c]
                    ey = elup.tile([P, 512], dt.float32, tag="ey")
                    nc.scalar.activation(ey, src, Act.Exp)
                    em = elup.tile([P, 512], dt.float32, tag="em")
                    nc.vector.tensor_scalar(
                        out=em, in0=ey, scalar1=1.0, scalar2=-1.0,
                        op0=Alu.min, op1=Alu.add,
                    )
                    nc.vector.scalar_tensor_tensor(
                        out=oT_sb[:, h, c * 512 : (c + 1) * 512],
                        in0=src, scalar=0.0, in1=em, op0=Alu.max, op1=Alu.add,
                    )
                nc.scalar.dma_start(
                    out=outh[h * F : (h + 1) * F, :], in_=oT_sb[:, h, :]
                )

    nc.compile()
    return nc


def kernel(x, weights, W, a, adj):
    global LAST_EXEC_NS
    from concourse.bass_utils import run_bass_kernel_spmd

    x = np.asarray(x, dtype=np.float32)
    weights = np.asarray(weights, dtype=np.float32)
    W = np.asarray(W, dtype=np.float32)
    a = np.asarray(a, dtype=np.float32)
    adj = np.asarray(adj, dtype=np.int32)

    with _BUILD_LOCK:
        if "nc" not in _CACHE:
            _CACHE["nc"] = _build_program()
    nc = _CACHE["nc"]

    # per-batch shards (shared across head-pair cores)
    sh = []
    for b in range(B):
        sh.append(
            {
                "adjTA": np.ascontiguousarray(adj[b, :U, U:].T),
                "adjTB": np.ascontiguousarray(adj[b, U:, :U].T),
                "wmA": np.ascontiguousarray(weights[b]),
                "wmB": np.ascontiguousarray(weights[b].T),
                "xloT": np.ascontiguousarray(x[b, :U].T),
                "xhiT": np.ascontiguousarray(x[b, U:].T),
            }
        )
    wcombs = []
    for hp in range(2):
        cm = np.empty((FIN, 2 * F + 2 * FIN + 4), dtype=np.float32)
        for h in range(2):
            cm[:, h * F : (h + 1) * F] = W[2 * hp + h]
            cm[:, 2 * F + h * FIN : 2 * F + (h + 1) * FIN] = W[2 * hp + h].T
            cm[:, 2 * F + 2 * FIN + 2 * h] = a[2 * hp + h, :F, 0]
            cm[:, 2 * F + 2 * FIN + 2 * h + 1] = a[2 * hp + h, F:, 0]
        wcombs.append(cm)

    in_maps = []
    for c in range(8):
        b, blk, hp = c // 4, (c // 2) % 2, c % 2
        s = sh[b]
        if blk == 0:  # block A: out rows [0,U), j-range = [U,2N)
            m = {"adjT": s["adjTA"], "wmx": s["wmA"], "xiT": s["xloT"], "xjT": s["xhiT"]}
        else:  # block B: out rows [U,2N), j-range = [0,U)
            m = {"adjT": s["adjTB"], "wmx": s["wmB"], "xiT": s["xhiT"], "xjT": s["xloT"]}
        m["wcomb"] = wcombs[hp]
        in_maps.append(m)

    res = run_bass_kernel_spmd(nc, in_maps, core_ids=list(range(8)), trace=TRACE)
    if res.exec_time_ns is not None:
        LAST_EXEC_NS = res.exec_time_ns

    out = np.empty((B, N, H * F), dtype=np.float32)
    for c in range(8):
        b, blk, hp = c // 4, (c // 2) % 2, c % 2
        r = np.asarray(res.results[c]["outh"], dtype=np.float32)  # [2F, U]
        blk_out = r.reshape(2, F, U).transpose(2, 0, 1).reshape(U, 2 * F)
        out[b, blk * U : (blk + 1) * U, hp * 2 * F : (hp + 1) * 2 * F] = blk_out
    return out
